# revision 10
# baseline (speedup 1.0000x reference)
"""Trainium2 Bass kernel for nn_DongTaiBaGuaZhen.

Reference math (B=4, S=8192, D=1024, HD=128, H=8 heads, P=32):
    heads[h]  = (x @ W_proj[h]) * cos(pi*freqs[h])
    hm        = heads.mean(S)                  -> tiny "impedance" net -> imp, coeff (B,8,8)
    mix[i]    = sum_g coeff[b,i,g] * heads[g]
    out_heads = heads + ((s+1)/S) * mix
    y         = LayerNorm(concat(out_heads) @ W_out + b_out + x) * gamma + beta

Key algebra: the S-mean commutes with the projection, so hm needs only
xbar = mean_s(x).  The whole main pass collapses to

    y_pre[b,s] = x[b,s] @ (Wt @ W_out + I)  +  cs(s) * (x[b,s] @ Wt @ G_b @ W_out) + b_out
    y = LN(y_pre) * gamma + beta

with Wt = concat_h(W_proj[h]*cos(pi*freqs[h])) (D,D) and G_b = coeff[b].T (x) I_HD.

Plan (8 NeuronCores, tokens (B*S = 32768) sharded contiguously, 4096/core,
so each core holds half of one batch's sequence):
  Launch 1 (device): per-core column sums of x  -> host: xbar_b, tiny net (exact
            fp32 numpy), imp output, A' = Wt@W_out + I, B_b = Wt@(G_b@W_out).
  Launch 2 (device): per 128-token group: u = x@A' and v = x@B_b via float32r
            matmuls (PSUM f32 accumulation), epilogue
            ypre = u + cs*v + b_out; LayerNorm via bn_stats; *gamma + beta; DMA out.
"""

import math
import numpy as np

import concourse.bacc as bacc
import concourse.mybir as mybir
import concourse.tile as tile
from concourse.bass_utils import run_bass_kernel_spmd

B, S, D, HD, P, H = 4, 8192, 1024, 128, 32, 8
EPS_LN = 1e-5
NCORES = 8
TOK = (B * S) // NCORES      # 4096 tokens per core
NT = TOK // 128              # 32 token groups of 128
CH = 512                     # free-dim width of x^T chunk tiles
NCH = TOK // CH              # 8 chunks
NDB = D // 128               # 8 din blocks
F32 = mybir.dt.float32
F32R = mybir.dt.float32r
CORE_IDS = list(range(NCORES))

_BUILT = {}
LAST_EXEC_NS = {}


def _build_stats_nc(tok=TOK, reps=1):
    """Launch 1: colsum[p, dblk] = sum_t xT[dblk*128+p, t] per core."""
    nc = bacc.Bacc("TRN2", target_bir_lowering=False, debug=False,
                   num_devices=NCORES)
    xT = nc.dram_tensor("xT", [D, tok], F32, kind="ExternalInput").ap()
    colsum = nc.dram_tensor("colsum", [128, NDB], F32,
                            kind="ExternalOutput").ap()
    with tile.TileContext(nc) as tc:
        with (
            tc.tile_pool(name="sb", bufs=1) as pool,
            tc.tile_pool(name="acc", bufs=1) as accp,
        ):
            fin = accp.tile([128, NDB], F32, tag="fin")
            for _rep in range(reps):
                for dblk in range(NDB):
                    t = pool.tile([128, tok], F32, tag=f"x{dblk}")
                    nc.sync.dma_start(
                        t[:], xT[dblk * 128:(dblk + 1) * 128, :])
                    nc.vector.tensor_reduce(
                        fin[:, dblk:dblk + 1], t[:],
                        axis=mybir.AxisListType.X, op=mybir.AluOpType.add)
            nc.sync.dma_start(colsum[:], fin[:])
    nc.compile()
    return nc


def _build_main_nc(tok=TOK, reps=1, triv=(False, False, False)):
    """Launch 2: y = LN(x@A' + cs*(x@B) + b_out) * gamma + beta.

    triv = (bout_is_zero, gamma_is_one, beta_is_zero): skip the matching
    epilogue ops (DVE is nearly co-critical with PE otherwise).  The host
    checks the actual input values and picks the variant; the general path
    still handles arbitrary b_out/gamma/beta.
    """
    nc = bacc.Bacc("TRN2", target_bir_lowering=False, debug=False,
                   num_devices=NCORES)
    nt = tok // 128
    xT = nc.dram_tensor("xT", [D, tok], F32R, kind="ExternalInput").ap()
    Ad = nc.dram_tensor("A", [D, D], F32R, kind="ExternalInput").ap()
    Bd = nc.dram_tensor("Bb", [D, D], F32R, kind="ExternalInput").ap()
    csd = nc.dram_tensor("cs", [128, nt], F32, kind="ExternalInput").ap()
    gbc = nc.dram_tensor("gamma_bc", [128, D], F32, kind="ExternalInput").ap()
    bbc = nc.dram_tensor("beta_bc", [128, D], F32, kind="ExternalInput").ap()
    obc = nc.dram_tensor("bout_bc", [128, D], F32, kind="ExternalInput").ap()
    yd = nc.dram_tensor("y", [tok, D], F32, kind="ExternalOutput").ap()

    with tile.TileContext(nc) as tc:
        with (
            tc.tile_pool(name="wp", bufs=1) as wp,
            tc.tile_pool(name="cp", bufs=1) as cp,
            tc.tile_pool(name="xp", bufs=2) as xp,
            tc.tile_pool(name="pp", bufs=2, space="PSUM") as pp,
            tc.tile_pool(name="op", bufs=3) as op,
            tc.tile_pool(name="sp", bufs=4) as sp,
        ):
            skip_bout, skip_gamma, skip_beta = triv
            At, Bt = [], []
            first_x = []
            for dblk in range(NDB):
                a = wp.tile([128, D], F32R, tag=f"A{dblk}")
                nc.sync.dma_start(a[:], Ad[dblk * 128:(dblk + 1) * 128, :])
                At.append(a)
                xt = xp.tile([128, CH], F32R, tag=f"x{dblk}")
                nc.sync.dma_start(xt[:], xT[dblk * 128:(dblk + 1) * 128,
                                            0:CH])
                first_x.append(xt)
            cst = cp.tile([128, nt], F32, tag="cs")
            nc.sync.dma_start(cst[:], csd[:])
            gt = bt = ot = None
            if not skip_gamma:
                gt = cp.tile([128, D], F32, tag="g")
                nc.sync.dma_start(gt[:], gbc[:])
            if not skip_beta:
                bt = cp.tile([128, D], F32, tag="b")
                nc.sync.dma_start(bt[:], bbc[:])
            if not skip_bout:
                ot = cp.tile([128, D], F32, tag="o")
                nc.sync.dma_start(ot[:], obc[:])
            eps = cp.tile([128, 1], F32, tag="eps")
            nc.vector.memset(eps[:], float(EPS_LN))

            xtiles = None
            for _rep in range(reps):
             for g in range(nt):
                c, sub = divmod(g, CH // 128)
                if sub == 0:
                    if c == 0 and first_x is not None:
                        xtiles = first_x
                        first_x = None
                    else:
                        xtiles = []
                        for dblk in range(NDB):
                            xt = xp.tile([128, CH], F32R, tag=f"x{dblk}")
                            nc.sync.dma_start(
                                xt[:], xT[dblk * 128:(dblk + 1) * 128,
                                          c * CH:(c + 1) * CH])
                            xtiles.append(xt)
                if not Bt:
                    # B loads after chunk-0 x so u-matmuls start sooner
                    for dblk in range(NDB):
                        b_ = wp.tile([128, D], F32R, tag=f"B{dblk}")
                        nc.sync.dma_start(
                            b_[:], Bd[dblk * 128:(dblk + 1) * 128, :])
                        Bt.append(b_)
                u = pp.tile([128, D], F32, tag="u")
                v = pp.tile([128, D], F32, tag="v")
                for dblk in range(NDB):
                    lhs = xtiles[dblk][:, sub * 128:(sub + 1) * 128]
                    st = dblk == 0
                    sp_ = dblk == NDB - 1
                    nc.tensor.matmul(u[:, 0:512], lhs, At[dblk][:, 0:512],
                                     start=st, stop=sp_)
                    nc.tensor.matmul(u[:, 512:1024], lhs,
                                     At[dblk][:, 512:1024], start=st, stop=sp_)
                    nc.tensor.matmul(v[:, 0:512], lhs, Bt[dblk][:, 0:512],
                                     start=st, stop=sp_)
                    nc.tensor.matmul(v[:, 512:1024], lhs,
                                     Bt[dblk][:, 512:1024], start=st, stop=sp_)

                # epilogue: ypre = u + cs*v + b_out ; LN ; *gamma + beta
                tmp = op.tile([128, D], F32, tag="tmp")
                nc.scalar.activation(tmp[:], v[:],
                                     mybir.ActivationFunctionType.Copy,
                                     scale=cst[:, g:g + 1])
                ypre = op.tile([128, D], F32, tag="ypre")
                nc.vector.tensor_tensor(ypre[:], tmp[:], u[:],
                                        op=mybir.AluOpType.add)
                if not skip_bout:
                    nc.vector.tensor_tensor(ypre[:], ypre[:], ot[:],
                                            op=mybir.AluOpType.add)
                st6 = sp.tile([128, 2, 6], F32, tag="st6")
                nc.vector.bn_stats(st6[:, 0, :], ypre[:, 0:512])
                nc.vector.bn_stats(st6[:, 1, :], ypre[:, 512:1024])
                mv = sp.tile([128, 2], F32, tag="mv")
                nc.vector.bn_aggr(mv[:], st6[:])
                std = sp.tile([128, 1], F32, tag="std")
                nc.scalar.activation(std[:], mv[:, 1:2],
                                     mybir.ActivationFunctionType.Sqrt,
                                     bias=eps[:])
                rstd = sp.tile([128, 1], F32, tag="rstd")
                nc.vector.reciprocal(rstd[:], std[:])
                yn = op.tile([128, D], F32, tag="yn")
                nc.vector.tensor_scalar(yn[:], ypre[:], mv[:, 0:1], rstd[:],
                                        op0=mybir.AluOpType.subtract,
                                        op1=mybir.AluOpType.mult)
                yo = yn
                if not skip_gamma:
                    yg = op.tile([128, D], F32, tag="yg")
                    nc.vector.tensor_tensor(yg[:], yo[:], gt[:],
                                            op=mybir.AluOpType.mult)
                    yo = yg
                if not skip_beta:
                    yb = op.tile([128, D], F32, tag="yb")
                    nc.vector.tensor_tensor(yb[:], yo[:], bt[:],
                                            op=mybir.AluOpType.add)
                    yo = yb
                nc.sync.dma_start(yd[g * 128:(g + 1) * 128, :], yo[:])
    nc.compile()
    return nc


def _get_nc(name, tok=TOK, triv=(False, False, False)):
    key = (name, tok, triv)
    if key not in _BUILT:
        if name == "stats":
            _BUILT[key] = _build_stats_nc(tok)
        else:
            _BUILT[key] = _build_main_nc(tok, triv=triv)
    return _BUILT[key]


def _erf(x):
    return np.vectorize(math.erf)(x).astype(np.float32)


def kernel(**inputs):
    x = np.asarray(inputs["x"], dtype=np.float32)
    W_proj = np.asarray(inputs["W_proj"], dtype=np.float32)
    freqs = np.asarray(inputs["freqs"], dtype=np.float32)
    W_pol = np.asarray(inputs["W_pol"], dtype=np.float32)
    b_pol = np.asarray(inputs["b_pol"], dtype=np.float32)
    W_imp1 = np.asarray(inputs["W_imp1"], dtype=np.float32)
    b_imp1 = np.asarray(inputs["b_imp1"], dtype=np.float32)
    W_imp2 = np.asarray(inputs["W_imp2"], dtype=np.float32)
    b_imp2 = np.asarray(inputs["b_imp2"], dtype=np.float32)
    W_out = np.asarray(inputs["W_out"], dtype=np.float32)
    b_out = np.asarray(inputs["b_out"], dtype=np.float32)
    gamma = np.asarray(inputs["gamma"], dtype=np.float32)
    beta = np.asarray(inputs["beta"], dtype=np.float32)

    # Wt[din, h*HD+e] = W_proj[h, din, e] * cos(pi*freqs[h, e])
    Wt = (W_proj * np.cos(np.pi * freqs)[:, None, :]).transpose(1, 0, 2)
    Wt = np.ascontiguousarray(Wt.reshape(D, D))
    A = Wt @ W_out + np.eye(D, dtype=np.float32)

    # per-core token shards: core c -> batch c//2, half c%2 (contiguous 4096)
    shards = []
    for c in range(NCORES):
        b, half = divmod(c, 2)
        shards.append(np.ascontiguousarray(
            x[b, half * TOK:(half + 1) * TOK, :].T))

    # ---- launch 1: column sums -> xbar ----
    stats_nc = _get_nc("stats")
    # exact host sums of 4 sampled dims as a corruption check
    chk_dims = [0, 257, 514, 1023]
    chk = {c: shards[c][chk_dims, :].sum(axis=1) for c in range(NCORES)}
    for attempt in range(3):
        res1 = run_bass_kernel_spmd(
            stats_nc, [{"xT": s} for s in shards], CORE_IDS)
        colsums = [res1.results[c]["colsum"].T.reshape(D)
                   for c in range(NCORES)]
        ok = all(
            np.allclose(colsums[c][chk_dims], chk[c], atol=2e-2)
            for c in range(NCORES))
        if ok:
            break
    else:
        raise RuntimeError("stats launch failed integrity check 3x")
    xbar = np.stack([(colsums[2 * b] + colsums[2 * b + 1]) / S
                     for b in range(B)])  # (B, D)

    # ---- host: tiny impedance net (exact fp32) ----
    hmc = xbar @ Wt                       # (B, D): hm[b, h*HD+e]
    hm = hmc.reshape(B, H, HD)
    pv = np.tanh(np.einsum('bhd,hdp->bhp', hm, W_pol) + b_pol[None])
    nrm = np.maximum(np.linalg.norm(pv, axis=-1, keepdims=True), 1e-12)
    pm = pv / nrm
    dp = np.einsum('bhp,bgp->bhg', pm, pm)
    zz = dp[..., None] * W_imp1[0] + b_imp1
    z = (0.5 * zz * (1.0 + _erf(zz / np.sqrt(2.0)))).astype(np.float32)
    pre = np.einsum('bhgk,ko->bhgo', z, W_imp2)[..., 0] + b_imp2[0]
    impv = (np.log1p(np.exp(-np.abs(pre))) + np.maximum(pre, 0.0))
    eye = np.eye(H, dtype=np.float32)
    imp = (impv * (1.0 - eye)).astype(np.float32)
    coeff = (0.1 / (1.0 + imp)) * (1.0 - eye)

    # B_b = Wt @ (G_b @ W_out);  (G_b@W_out)[g*HD+e,:] = sum_i c[b,i,g] Wo4[i,e,:]
    Wo4 = W_out.reshape(H, HD, D)
    Bmats = []
    for b in range(B):
        Mb = np.einsum('ig,ied->ged', coeff[b], Wo4).reshape(D, D)
        Bmats.append(np.ascontiguousarray(Wt @ Mb))

    # ---- launch 2: main pass ----
    cs_all = (np.arange(S, dtype=np.float32) + 1.0) / S
    gamma_bc = np.ascontiguousarray(np.broadcast_to(gamma, (128, D)))
    beta_bc = np.ascontiguousarray(np.broadcast_to(beta, (128, D)))
    bout_bc = np.ascontiguousarray(np.broadcast_to(b_out, (128, D)))
    in_maps = []
    for c in range(NCORES):
        b, half = divmod(c, 2)
        cs_c = cs_all[half * TOK:(half + 1) * TOK]
        cs_t = np.ascontiguousarray(cs_c.reshape(NT, 128).T)
        in_maps.append({
            "xT": shards[c], "A": A, "Bb": Bmats[b], "cs": cs_t,
            "gamma_bc": gamma_bc, "beta_bc": beta_bc, "bout_bc": bout_bc,
        })
    triv = (not b_out.any(), bool((gamma == 1.0).all()), not beta.any())
    main_nc = _get_nc("main", triv=triv)
    # LN-output invariant for a corruption check: sampled rows must have
    # mean ~ mean(beta) and second moment ~ mean(gamma^2 + beta^2), loosely.
    m_lo = 0.2 * float((gamma ** 2 + beta ** 2).mean()) + 1e-6
    m_hi = 5.0 * float((gamma ** 2 + beta ** 2).mean()) + 1e-6
    rows = np.linspace(0, TOK - 1, 16).astype(int)
    for attempt in range(3):
        res2 = run_bass_kernel_spmd(main_nc, in_maps, CORE_IDS)
        ok = True
        for c in range(NCORES):
            yr = res2.results[c]["y"][rows]
            if not np.isfinite(yr).all():
                ok = False
                break
            m2 = float((yr.astype(np.float64) ** 2).mean())
            if not (m_lo <= m2 <= m_hi):
                ok = False
                break
        if ok:
            break
    else:
        raise RuntimeError("main launch failed integrity check 3x")

    y = np.empty((B, S, D), dtype=np.float32)
    for c in range(NCORES):
        b, half = divmod(c, 2)
        y[b, half * TOK:(half + 1) * TOK, :] = res2.results[c]["y"]
    return y, imp


# revision 11
# speedup vs baseline: 1.0258x; 1.0258x over previous
"""Trainium2 Bass kernel for nn_DongTaiBaGuaZhen.

Reference math (B=4, S=8192, D=1024, HD=128, H=8 heads, P=32):
    heads[h]  = (x @ W_proj[h]) * cos(pi*freqs[h])
    hm        = heads.mean(S)                  -> tiny "impedance" net -> imp, coeff (B,8,8)
    mix[i]    = sum_g coeff[b,i,g] * heads[g]
    out_heads = heads + ((s+1)/S) * mix
    y         = LayerNorm(concat(out_heads) @ W_out + b_out + x) * gamma + beta

Key algebra: the S-mean commutes with the projection, so hm needs only
xbar = mean_s(x).  The whole main pass collapses to

    y_pre[b,s] = x[b,s] @ (Wt @ W_out + I)  +  cs(s) * (x[b,s] @ Wt @ G_b @ W_out) + b_out
    y = LN(y_pre) * gamma + beta

with Wt = concat_h(W_proj[h]*cos(pi*freqs[h])) (D,D) and G_b = coeff[b].T (x) I_HD.

Plan (8 NeuronCores, tokens (B*S = 32768) sharded contiguously, 4096/core,
so each core holds half of one batch's sequence):
  Launch 1 (device): per-core column sums of x  -> host: xbar_b, tiny net (exact
            fp32 numpy), imp output, A' = Wt@W_out + I, B_b = Wt@(G_b@W_out).
  Launch 2 (device): per 128-token group: u = x@A' and v = x@B_b via float32r
            matmuls (PSUM f32 accumulation), epilogue
            ypre = u + cs*v + b_out; LayerNorm via bn_stats; *gamma + beta; DMA out.
"""

import math
import time

import numpy as np

import concourse.bacc as bacc
import concourse.mybir as mybir
import concourse.tile as tile
from concourse.bass_utils import run_bass_kernel_spmd

B, S, D, HD, P, H = 4, 8192, 1024, 128, 32, 8
EPS_LN = 1e-5
NCORES = 8
TOK = (B * S) // NCORES      # 4096 tokens per core
NT = TOK // 128              # 32 token groups of 128
CH = 512                     # free-dim width of x^T chunk tiles
NCH = TOK // CH              # 8 chunks
NDB = D // 128               # 8 din blocks
F32 = mybir.dt.float32
F32R = mybir.dt.float32r
CORE_IDS = list(range(NCORES))

_BUILT = {}
LAST_EXEC_NS = {}


def _build_stats_nc(tok=TOK, reps=1):
    """Launch 1: colsum[p, dblk] = sum_t xT[dblk*128+p, t] per core."""
    nc = bacc.Bacc("TRN2", target_bir_lowering=False, debug=False,
                   num_devices=NCORES)
    xT = nc.dram_tensor("xT", [D, tok], F32, kind="ExternalInput").ap()
    colsum = nc.dram_tensor("colsum", [128, NDB], F32,
                            kind="ExternalOutput").ap()
    with tile.TileContext(nc) as tc:
        with (
            tc.tile_pool(name="sb", bufs=1) as pool,
            tc.tile_pool(name="acc", bufs=1) as accp,
        ):
            fin = accp.tile([128, NDB], F32, tag="fin")
            for _rep in range(reps):
                for dblk in range(NDB):
                    t = pool.tile([128, tok], F32, tag=f"x{dblk}")
                    nc.sync.dma_start(
                        t[:], xT[dblk * 128:(dblk + 1) * 128, :])
                    nc.vector.tensor_reduce(
                        fin[:, dblk:dblk + 1], t[:],
                        axis=mybir.AxisListType.X, op=mybir.AluOpType.add)
            nc.sync.dma_start(colsum[:], fin[:])
    nc.compile()
    return nc


def _build_main_nc(tok=TOK, reps=1, triv=(False, False, False)):
    """Launch 2: y = LN(x@A' + cs*(x@B) + b_out) * gamma + beta.

    triv = (bout_is_zero, gamma_is_one, beta_is_zero): skip the matching
    epilogue ops (DVE is nearly co-critical with PE otherwise).  The host
    checks the actual input values and picks the variant; the general path
    still handles arbitrary b_out/gamma/beta.
    """
    nc = bacc.Bacc("TRN2", target_bir_lowering=False, debug=False,
                   num_devices=NCORES)
    nt = tok // 128
    xT = nc.dram_tensor("xT", [D, tok], F32R, kind="ExternalInput").ap()
    Ad = nc.dram_tensor("A", [D, D], F32R, kind="ExternalInput").ap()
    Bd = nc.dram_tensor("Bb", [D, D], F32R, kind="ExternalInput").ap()
    csd = nc.dram_tensor("cs", [128, nt], F32, kind="ExternalInput").ap()
    gbc = nc.dram_tensor("gamma_bc", [128, D], F32, kind="ExternalInput").ap()
    bbc = nc.dram_tensor("beta_bc", [128, D], F32, kind="ExternalInput").ap()
    obc = nc.dram_tensor("bout_bc", [128, D], F32, kind="ExternalInput").ap()
    yd = nc.dram_tensor("y", [tok, D], F32, kind="ExternalOutput").ap()

    with tile.TileContext(nc) as tc:
        with (
            tc.tile_pool(name="wp", bufs=1) as wp,
            tc.tile_pool(name="cp", bufs=1) as cp,
            tc.tile_pool(name="xp", bufs=2) as xp,
            tc.tile_pool(name="pp", bufs=2, space="PSUM") as pp,
            tc.tile_pool(name="op", bufs=3) as op,
            tc.tile_pool(name="sp", bufs=4) as sp,
        ):
            skip_bout, skip_gamma, skip_beta = triv
            At, Bt = [], []
            first_x = []
            for dblk in range(NDB):
                a = wp.tile([128, D], F32R, tag=f"A{dblk}")
                nc.sync.dma_start(a[:], Ad[dblk * 128:(dblk + 1) * 128, :])
                At.append(a)
                xt = xp.tile([128, CH], F32R, tag=f"x{dblk}")
                nc.sync.dma_start(xt[:], xT[dblk * 128:(dblk + 1) * 128,
                                            0:CH])
                first_x.append(xt)
            cst = cp.tile([128, nt], F32, tag="cs")
            nc.sync.dma_start(cst[:], csd[:])
            gt = bt = ot = None
            if not skip_gamma:
                gt = cp.tile([128, D], F32, tag="g")
                nc.sync.dma_start(gt[:], gbc[:])
            if not skip_beta:
                bt = cp.tile([128, D], F32, tag="b")
                nc.sync.dma_start(bt[:], bbc[:])
            if not skip_bout:
                ot = cp.tile([128, D], F32, tag="o")
                nc.sync.dma_start(ot[:], obc[:])
            eps = cp.tile([128, 1], F32, tag="eps")
            nc.vector.memset(eps[:], float(EPS_LN))

            xtiles = None
            for _rep in range(reps):
             for g in range(nt):
                c, sub = divmod(g, CH // 128)
                if sub == 0:
                    if c == 0 and first_x is not None:
                        xtiles = first_x
                        first_x = None
                    else:
                        xtiles = []
                        for dblk in range(NDB):
                            xt = xp.tile([128, CH], F32R, tag=f"x{dblk}")
                            nc.sync.dma_start(
                                xt[:], xT[dblk * 128:(dblk + 1) * 128,
                                          c * CH:(c + 1) * CH])
                            xtiles.append(xt)
                if not Bt:
                    # B loads after chunk-0 x so u-matmuls start sooner
                    for dblk in range(NDB):
                        b_ = wp.tile([128, D], F32R, tag=f"B{dblk}")
                        nc.sync.dma_start(
                            b_[:], Bd[dblk * 128:(dblk + 1) * 128, :])
                        Bt.append(b_)
                u = pp.tile([128, D], F32, tag="u")
                v = pp.tile([128, D], F32, tag="v")
                for dblk in range(NDB):
                    lhs = xtiles[dblk][:, sub * 128:(sub + 1) * 128]
                    st = dblk == 0
                    sp_ = dblk == NDB - 1
                    nc.tensor.matmul(u[:, 0:512], lhs, At[dblk][:, 0:512],
                                     start=st, stop=sp_)
                    nc.tensor.matmul(u[:, 512:1024], lhs,
                                     At[dblk][:, 512:1024], start=st, stop=sp_)
                    nc.tensor.matmul(v[:, 0:512], lhs, Bt[dblk][:, 0:512],
                                     start=st, stop=sp_)
                    nc.tensor.matmul(v[:, 512:1024], lhs,
                                     Bt[dblk][:, 512:1024], start=st, stop=sp_)

                # epilogue: ypre = u + cs*v + b_out ; LN ; *gamma + beta
                tmp = op.tile([128, D], F32, tag="tmp")
                nc.scalar.activation(tmp[:], v[:],
                                     mybir.ActivationFunctionType.Copy,
                                     scale=cst[:, g:g + 1])
                ypre = op.tile([128, D], F32, tag="ypre")
                nc.vector.tensor_tensor(ypre[:], tmp[:], u[:],
                                        op=mybir.AluOpType.add)
                if not skip_bout:
                    nc.vector.tensor_tensor(ypre[:], ypre[:], ot[:],
                                            op=mybir.AluOpType.add)
                st6 = sp.tile([128, 2, 6], F32, tag="st6")
                nc.vector.bn_stats(st6[:, 0, :], ypre[:, 0:512])
                nc.vector.bn_stats(st6[:, 1, :], ypre[:, 512:1024])
                mv = sp.tile([128, 2], F32, tag="mv")
                nc.vector.bn_aggr(mv[:], st6[:])
                std = sp.tile([128, 1], F32, tag="std")
                nc.scalar.activation(std[:], mv[:, 1:2],
                                     mybir.ActivationFunctionType.Sqrt,
                                     bias=eps[:])
                rstd = sp.tile([128, 1], F32, tag="rstd")
                nc.vector.reciprocal(rstd[:], std[:])
                yn = op.tile([128, D], F32, tag="yn")
                nc.vector.tensor_scalar(yn[:], ypre[:], mv[:, 0:1], rstd[:],
                                        op0=mybir.AluOpType.subtract,
                                        op1=mybir.AluOpType.mult)
                yo = yn
                if not skip_gamma:
                    yg = op.tile([128, D], F32, tag="yg")
                    nc.vector.tensor_tensor(yg[:], yo[:], gt[:],
                                            op=mybir.AluOpType.mult)
                    yo = yg
                if not skip_beta:
                    yb = op.tile([128, D], F32, tag="yb")
                    nc.vector.tensor_tensor(yb[:], yo[:], bt[:],
                                            op=mybir.AluOpType.add)
                    yo = yb
                nc.sync.dma_start(yd[g * 128:(g + 1) * 128, :], yo[:])
    nc.compile()
    return nc


def _get_nc(name, tok=TOK, triv=(False, False, False)):
    key = (name, tok, triv)
    if key not in _BUILT:
        if name == "stats":
            _BUILT[key] = _build_stats_nc(tok)
        else:
            _BUILT[key] = _build_main_nc(tok, triv=triv)
    return _BUILT[key]


def _erf(x):
    return np.vectorize(math.erf)(x).astype(np.float32)


def kernel(**inputs):
    x = np.asarray(inputs["x"], dtype=np.float32)
    W_proj = np.asarray(inputs["W_proj"], dtype=np.float32)
    freqs = np.asarray(inputs["freqs"], dtype=np.float32)
    W_pol = np.asarray(inputs["W_pol"], dtype=np.float32)
    b_pol = np.asarray(inputs["b_pol"], dtype=np.float32)
    W_imp1 = np.asarray(inputs["W_imp1"], dtype=np.float32)
    b_imp1 = np.asarray(inputs["b_imp1"], dtype=np.float32)
    W_imp2 = np.asarray(inputs["W_imp2"], dtype=np.float32)
    b_imp2 = np.asarray(inputs["b_imp2"], dtype=np.float32)
    W_out = np.asarray(inputs["W_out"], dtype=np.float32)
    b_out = np.asarray(inputs["b_out"], dtype=np.float32)
    gamma = np.asarray(inputs["gamma"], dtype=np.float32)
    beta = np.asarray(inputs["beta"], dtype=np.float32)

    # Wt[din, h*HD+e] = W_proj[h, din, e] * cos(pi*freqs[h, e])
    Wt = (W_proj * np.cos(np.pi * freqs)[:, None, :]).transpose(1, 0, 2)
    Wt = np.ascontiguousarray(Wt.reshape(D, D))
    A = Wt @ W_out + np.eye(D, dtype=np.float32)

    # per-core token shards: core c -> batch c//2, half c%2 (contiguous 4096)
    shards = []
    for c in range(NCORES):
        b, half = divmod(c, 2)
        shards.append(np.ascontiguousarray(
            x[b, half * TOK:(half + 1) * TOK, :].T))

    # ---- launch 1: column sums -> xbar ----
    stats_nc = _get_nc("stats")
    # exact host sums of 4 sampled dims as a corruption check
    chk_dims = [0, 257, 514, 1023]
    chk = {c: shards[c][chk_dims, :].sum(axis=1) for c in range(NCORES)}
    last_exc = None
    for attempt in range(3):
        try:
            res1 = run_bass_kernel_spmd(
                stats_nc, [{"xT": s} for s in shards], CORE_IDS)
            colsums = [res1.results[c]["colsum"].T.reshape(D)
                       for c in range(NCORES)]
        except Exception as e:
            last_exc = e
            time.sleep(2.0)
            continue
        ok = all(
            np.allclose(colsums[c][chk_dims], chk[c], atol=2e-2)
            for c in range(NCORES))
        if ok:
            break
    else:
        raise RuntimeError(
            f"stats launch failed integrity check 3x (last exc: {last_exc})")
    xbar = np.stack([(colsums[2 * b] + colsums[2 * b + 1]) / S
                     for b in range(B)])  # (B, D)

    # ---- host: tiny impedance net (exact fp32) ----
    hmc = xbar @ Wt                       # (B, D): hm[b, h*HD+e]
    hm = hmc.reshape(B, H, HD)
    pv = np.tanh(np.einsum('bhd,hdp->bhp', hm, W_pol) + b_pol[None])
    nrm = np.maximum(np.linalg.norm(pv, axis=-1, keepdims=True), 1e-12)
    pm = pv / nrm
    dp = np.einsum('bhp,bgp->bhg', pm, pm)
    zz = dp[..., None] * W_imp1[0] + b_imp1
    z = (0.5 * zz * (1.0 + _erf(zz / np.sqrt(2.0)))).astype(np.float32)
    pre = np.einsum('bhgk,ko->bhgo', z, W_imp2)[..., 0] + b_imp2[0]
    impv = (np.log1p(np.exp(-np.abs(pre))) + np.maximum(pre, 0.0))
    eye = np.eye(H, dtype=np.float32)
    imp = (impv * (1.0 - eye)).astype(np.float32)
    coeff = (0.1 / (1.0 + imp)) * (1.0 - eye)

    # B_b = Wt @ (G_b @ W_out);  (G_b@W_out)[g*HD+e,:] = sum_i c[b,i,g] Wo4[i,e,:]
    Wo4 = W_out.reshape(H, HD, D)
    Bmats = []
    for b in range(B):
        Mb = np.einsum('ig,ied->ged', coeff[b], Wo4).reshape(D, D)
        Bmats.append(np.ascontiguousarray(Wt @ Mb))

    # ---- launch 2: main pass ----
    cs_all = (np.arange(S, dtype=np.float32) + 1.0) / S
    gamma_bc = np.ascontiguousarray(np.broadcast_to(gamma, (128, D)))
    beta_bc = np.ascontiguousarray(np.broadcast_to(beta, (128, D)))
    bout_bc = np.ascontiguousarray(np.broadcast_to(b_out, (128, D)))
    in_maps = []
    for c in range(NCORES):
        b, half = divmod(c, 2)
        cs_c = cs_all[half * TOK:(half + 1) * TOK]
        cs_t = np.ascontiguousarray(cs_c.reshape(NT, 128).T)
        in_maps.append({
            "xT": shards[c], "A": A, "Bb": Bmats[b], "cs": cs_t,
            "gamma_bc": gamma_bc, "beta_bc": beta_bc, "bout_bc": bout_bc,
        })
    triv = (not b_out.any(), bool((gamma == 1.0).all()), not beta.any())
    main_nc = _get_nc("main", triv=triv)
    # LN-output invariant for a corruption check: sampled rows must have
    # mean ~ mean(beta) and second moment ~ mean(gamma^2 + beta^2), loosely.
    m_lo = 0.2 * float((gamma ** 2 + beta ** 2).mean()) + 1e-6
    m_hi = 5.0 * float((gamma ** 2 + beta ** 2).mean()) + 1e-6
    rows = np.linspace(0, TOK - 1, 16).astype(int)
    last_exc = None
    for attempt in range(3):
        try:
            res2 = run_bass_kernel_spmd(main_nc, in_maps, CORE_IDS)
        except Exception as e:
            last_exc = e
            time.sleep(2.0)
            continue
        ok = True
        for c in range(NCORES):
            yr = res2.results[c]["y"][rows]
            if not np.isfinite(yr).all():
                ok = False
                break
            m2 = float((yr.astype(np.float64) ** 2).mean())
            if not (m_lo <= m2 <= m_hi):
                ok = False
                break
        if ok:
            break
    else:
        raise RuntimeError(
            f"main launch failed integrity check 3x (last exc: {last_exc})")

    y = np.empty((B, S, D), dtype=np.float32)
    for c in range(NCORES):
        b, half = divmod(c, 2)
        y[b, half * TOK:(half + 1) * TOK, :] = res2.results[c]["y"]
    return y, imp


# revision 15
# speedup vs baseline: 1.0359x; 1.0099x over previous
"""Trainium2 Bass kernel for nn_DongTaiBaGuaZhen.

Reference math (B=4, S=8192, D=1024, HD=128, H=8 heads, P=32):
    heads[h]  = (x @ W_proj[h]) * cos(pi*freqs[h])
    hm        = heads.mean(S)                  -> tiny "impedance" net -> imp, coeff (B,8,8)
    mix[i]    = sum_g coeff[b,i,g] * heads[g]
    out_heads = heads + ((s+1)/S) * mix
    y         = LayerNorm(concat(out_heads) @ W_out + b_out + x) * gamma + beta

Key algebra: the S-mean commutes with the projection, so hm needs only
xbar = mean_s(x).  The whole main pass collapses to

    y_pre[b,s] = x[b,s] @ (Wt @ W_out + I)  +  cs(s) * (x[b,s] @ Wt @ G_b @ W_out) + b_out
    y = LN(y_pre) * gamma + beta

with Wt = concat_h(W_proj[h]*cos(pi*freqs[h])) (D,D) and G_b = coeff[b].T (x) I_HD.

Plan (8 NeuronCores, tokens (B*S = 32768) sharded contiguously, 4096/core,
so each core holds half of one batch's sequence):
  Launch 1 (device): per-core column sums of x  -> host: xbar_b, tiny net (exact
            fp32 numpy), imp output, A' = Wt@W_out + I, B_b = Wt@(G_b@W_out).
  Launch 2 (device): per 128-token group: u = x@A' and v = x@B_b via float32r
            matmuls (PSUM f32 accumulation), epilogue
            ypre = u + cs*v + b_out; LayerNorm via bn_stats; *gamma + beta; DMA out.
"""

import math
import time

import numpy as np

import concourse.bacc as bacc
import concourse.mybir as mybir
import concourse.tile as tile
from concourse.bass_utils import run_bass_kernel_spmd

B, S, D, HD, P, H = 4, 8192, 1024, 128, 32, 8
EPS_LN = 1e-5
NCORES = 8
TOK = (B * S) // NCORES      # 4096 tokens per core
NT = TOK // 128              # 32 token groups of 128
CH = 512                     # free-dim width of x^T chunk tiles
NCH = TOK // CH              # 8 chunks
NDB = D // 128               # 8 din blocks
F32 = mybir.dt.float32
F32R = mybir.dt.float32r
CORE_IDS = list(range(NCORES))

_BUILT = {}
LAST_EXEC_NS = {}


def _build_stats_nc(tok=TOK, reps=1):
    """Launch 1: colsum[p, dblk] = sum_t xT[dblk*128+p, t] per core."""
    nc = bacc.Bacc("TRN2", target_bir_lowering=False, debug=False,
                   num_devices=NCORES)
    xT = nc.dram_tensor("xT", [D, tok], F32, kind="ExternalInput").ap()
    colsum = nc.dram_tensor("colsum", [128, NDB], F32,
                            kind="ExternalOutput").ap()
    with tile.TileContext(nc) as tc:
        with (
            tc.tile_pool(name="sb", bufs=1) as pool,
            tc.tile_pool(name="acc", bufs=1) as accp,
        ):
            half = tok // 2
            acc = accp.tile([128, NDB, 2], F32, tag="acc")
            fin = accp.tile([128, NDB], F32, tag="fin")
            for _rep in range(reps):
                for dblk in range(NDB):
                    for hh in range(2):
                        t = pool.tile([128, half], F32, tag=f"x{dblk}_{hh}")
                        nc.sync.dma_start(
                            t[:], xT[dblk * 128:(dblk + 1) * 128,
                                     hh * half:(hh + 1) * half])
                        nc.vector.tensor_reduce(
                            acc[:, dblk, hh:hh + 1], t[:],
                            axis=mybir.AxisListType.X, op=mybir.AluOpType.add)
            nc.vector.tensor_reduce(
                fin[:], acc[:], axis=mybir.AxisListType.X,
                op=mybir.AluOpType.add)
            nc.sync.dma_start(colsum[:], fin[:])
    nc.compile()
    return nc


def _build_main_nc(tok=TOK, reps=1, triv=(False, False, False)):
    """Launch 2: y = LN(x@A' + cs*(x@B) + b_out) * gamma + beta.

    triv = (bout_is_zero, gamma_is_one, beta_is_zero): skip the matching
    epilogue ops (DVE is nearly co-critical with PE otherwise).  The host
    checks the actual input values and picks the variant; the general path
    still handles arbitrary b_out/gamma/beta.
    """
    nc = bacc.Bacc("TRN2", target_bir_lowering=False, debug=False,
                   num_devices=NCORES)
    nt = tok // 128
    xT = nc.dram_tensor("xT", [D, tok], F32R, kind="ExternalInput").ap()
    Ad = nc.dram_tensor("A", [D, D], F32R, kind="ExternalInput").ap()
    Bd = nc.dram_tensor("Bb", [D, D], F32R, kind="ExternalInput").ap()
    csd = nc.dram_tensor("cs", [128, nt], F32, kind="ExternalInput").ap()
    gbc = nc.dram_tensor("gamma_bc", [128, D], F32, kind="ExternalInput").ap()
    bbc = nc.dram_tensor("beta_bc", [128, D], F32, kind="ExternalInput").ap()
    obc = nc.dram_tensor("bout_bc", [128, D], F32, kind="ExternalInput").ap()
    yd = nc.dram_tensor("y", [tok, D], F32, kind="ExternalOutput").ap()

    with tile.TileContext(nc) as tc:
        with (
            tc.tile_pool(name="wp", bufs=1) as wp,
            tc.tile_pool(name="cp", bufs=1) as cp,
            tc.tile_pool(name="xp", bufs=2) as xp,
            tc.tile_pool(name="pp", bufs=2, space="PSUM") as pp,
            tc.tile_pool(name="op", bufs=3) as op,
            tc.tile_pool(name="sp", bufs=4) as sp,
        ):
            skip_bout, skip_gamma, skip_beta = triv
            At, Bt = [], []
            first_x = []
            for dblk in range(NDB):
                a = wp.tile([128, D], F32R, tag=f"A{dblk}")
                nc.sync.dma_start(a[:], Ad[dblk * 128:(dblk + 1) * 128, :])
                At.append(a)
                xt = xp.tile([128, CH], F32R, tag=f"x{dblk}")
                nc.sync.dma_start(xt[:], xT[dblk * 128:(dblk + 1) * 128,
                                            0:CH])
                first_x.append(xt)
            cst = cp.tile([128, nt], F32, tag="cs")
            nc.sync.dma_start(cst[:], csd[:])
            gt = bt = ot = None
            if not skip_gamma:
                gt = cp.tile([128, D], F32, tag="g")
                nc.sync.dma_start(gt[:], gbc[:])
            if not skip_beta:
                bt = cp.tile([128, D], F32, tag="b")
                nc.sync.dma_start(bt[:], bbc[:])
            if not skip_bout:
                ot = cp.tile([128, D], F32, tag="o")
                nc.sync.dma_start(ot[:], obc[:])
            eps = cp.tile([128, 1], F32, tag="eps")
            nc.vector.memset(eps[:], float(EPS_LN))

            xtiles = None
            for _rep in range(reps):
             for g in range(nt):
                c, sub = divmod(g, CH // 128)
                if sub == 0:
                    if c == 0 and first_x is not None:
                        xtiles = first_x
                        first_x = None
                    else:
                        xtiles = []
                        for dblk in range(NDB):
                            xt = xp.tile([128, CH], F32R, tag=f"x{dblk}")
                            nc.sync.dma_start(
                                xt[:], xT[dblk * 128:(dblk + 1) * 128,
                                          c * CH:(c + 1) * CH])
                            xtiles.append(xt)
                if not Bt:
                    # B loads after chunk-0 x so u-matmuls start sooner
                    for dblk in range(NDB):
                        b_ = wp.tile([128, D], F32R, tag=f"B{dblk}")
                        nc.sync.dma_start(
                            b_[:], Bd[dblk * 128:(dblk + 1) * 128, :])
                        Bt.append(b_)
                u = pp.tile([128, D], F32, tag="u")
                v = pp.tile([128, D], F32, tag="v")
                for dblk in range(NDB):
                    lhs = xtiles[dblk][:, sub * 128:(sub + 1) * 128]
                    st = dblk == 0
                    sp_ = dblk == NDB - 1
                    nc.tensor.matmul(u[:, 0:512], lhs, At[dblk][:, 0:512],
                                     start=st, stop=sp_)
                    nc.tensor.matmul(u[:, 512:1024], lhs,
                                     At[dblk][:, 512:1024], start=st, stop=sp_)
                    nc.tensor.matmul(v[:, 0:512], lhs, Bt[dblk][:, 0:512],
                                     start=st, stop=sp_)
                    nc.tensor.matmul(v[:, 512:1024], lhs,
                                     Bt[dblk][:, 512:1024], start=st, stop=sp_)

                # epilogue: ypre = u + cs*v + b_out ; LN ; *gamma + beta
                tmp = op.tile([128, D], F32, tag="tmp")
                nc.scalar.activation(tmp[:], v[:],
                                     mybir.ActivationFunctionType.Copy,
                                     scale=cst[:, g:g + 1])
                ypre = op.tile([128, D], F32, tag="ypre")
                nc.vector.tensor_tensor(ypre[:], tmp[:], u[:],
                                        op=mybir.AluOpType.add)
                if not skip_bout:
                    nc.vector.tensor_tensor(ypre[:], ypre[:], ot[:],
                                            op=mybir.AluOpType.add)
                st6 = sp.tile([128, 2, 6], F32, tag="st6")
                nc.vector.bn_stats(st6[:, 0, :], ypre[:, 0:512])
                nc.vector.bn_stats(st6[:, 1, :], ypre[:, 512:1024])
                mv = sp.tile([128, 2], F32, tag="mv")
                nc.vector.bn_aggr(mv[:], st6[:])
                mean_ap = mv[:, 0:1]
                std = sp.tile([128, 1], F32, tag="std")
                nc.scalar.activation(std[:], mv[:, 1:2],
                                     mybir.ActivationFunctionType.Sqrt,
                                     bias=eps[:])
                rstd = sp.tile([128, 1], F32, tag="rstd")
                nc.vector.reciprocal(rstd[:], std[:])
                yn = op.tile([128, D], F32, tag="yn")
                nc.vector.tensor_scalar(yn[:], ypre[:], mean_ap, rstd[:],
                                        op0=mybir.AluOpType.subtract,
                                        op1=mybir.AluOpType.mult)
                yo = yn
                if not skip_gamma:
                    yg = op.tile([128, D], F32, tag="yg")
                    nc.vector.tensor_tensor(yg[:], yo[:], gt[:],
                                            op=mybir.AluOpType.mult)
                    yo = yg
                if not skip_beta:
                    yb = op.tile([128, D], F32, tag="yb")
                    nc.vector.tensor_tensor(yb[:], yo[:], bt[:],
                                            op=mybir.AluOpType.add)
                    yo = yb
                nc.sync.dma_start(yd[g * 128:(g + 1) * 128, :], yo[:])
    nc.compile()
    return nc


def _get_nc(name, tok=TOK, triv=(False, False, False)):
    key = (name, tok, triv)
    if key not in _BUILT:
        if name == "stats":
            _BUILT[key] = _build_stats_nc(tok)
        else:
            _BUILT[key] = _build_main_nc(tok, triv=triv)
    return _BUILT[key]


def _erf(x):
    return np.vectorize(math.erf)(x).astype(np.float32)


def kernel(**inputs):
    x = np.asarray(inputs["x"], dtype=np.float32)
    W_proj = np.asarray(inputs["W_proj"], dtype=np.float32)
    freqs = np.asarray(inputs["freqs"], dtype=np.float32)
    W_pol = np.asarray(inputs["W_pol"], dtype=np.float32)
    b_pol = np.asarray(inputs["b_pol"], dtype=np.float32)
    W_imp1 = np.asarray(inputs["W_imp1"], dtype=np.float32)
    b_imp1 = np.asarray(inputs["b_imp1"], dtype=np.float32)
    W_imp2 = np.asarray(inputs["W_imp2"], dtype=np.float32)
    b_imp2 = np.asarray(inputs["b_imp2"], dtype=np.float32)
    W_out = np.asarray(inputs["W_out"], dtype=np.float32)
    b_out = np.asarray(inputs["b_out"], dtype=np.float32)
    gamma = np.asarray(inputs["gamma"], dtype=np.float32)
    beta = np.asarray(inputs["beta"], dtype=np.float32)

    # Wt[din, h*HD+e] = W_proj[h, din, e] * cos(pi*freqs[h, e])
    Wt = (W_proj * np.cos(np.pi * freqs)[:, None, :]).transpose(1, 0, 2)
    Wt = np.ascontiguousarray(Wt.reshape(D, D))
    A = Wt @ W_out + np.eye(D, dtype=np.float32)

    # per-core token shards: core c -> batch c//2, half c%2 (contiguous 4096)
    shards = []
    for c in range(NCORES):
        b, half = divmod(c, 2)
        shards.append(np.ascontiguousarray(
            x[b, half * TOK:(half + 1) * TOK, :].T))

    # ---- launch 1: column sums -> xbar ----
    stats_nc = _get_nc("stats")
    # exact host sums of 4 sampled dims as a corruption check
    chk_dims = [0, 257, 514, 1023]
    chk = {c: shards[c][chk_dims, :].sum(axis=1) for c in range(NCORES)}
    last_exc = None
    for attempt in range(3):
        try:
            res1 = run_bass_kernel_spmd(
                stats_nc, [{"xT": s} for s in shards], CORE_IDS)
            colsums = [res1.results[c]["colsum"].T.reshape(D)
                       for c in range(NCORES)]
        except Exception as e:
            last_exc = e
            time.sleep(2.0)
            continue
        ok = all(
            np.allclose(colsums[c][chk_dims], chk[c], atol=2e-2)
            for c in range(NCORES))
        if ok:
            break
    else:
        raise RuntimeError(
            f"stats launch failed integrity check 3x (last exc: {last_exc})")
    xbar = np.stack([(colsums[2 * b] + colsums[2 * b + 1]) / S
                     for b in range(B)])  # (B, D)

    # ---- host: tiny impedance net (exact fp32) ----
    hmc = xbar @ Wt                       # (B, D): hm[b, h*HD+e]
    hm = hmc.reshape(B, H, HD)
    pv = np.tanh(np.einsum('bhd,hdp->bhp', hm, W_pol) + b_pol[None])
    nrm = np.maximum(np.linalg.norm(pv, axis=-1, keepdims=True), 1e-12)
    pm = pv / nrm
    dp = np.einsum('bhp,bgp->bhg', pm, pm)
    zz = dp[..., None] * W_imp1[0] + b_imp1
    z = (0.5 * zz * (1.0 + _erf(zz / np.sqrt(2.0)))).astype(np.float32)
    pre = np.einsum('bhgk,ko->bhgo', z, W_imp2)[..., 0] + b_imp2[0]
    impv = (np.log1p(np.exp(-np.abs(pre))) + np.maximum(pre, 0.0))
    eye = np.eye(H, dtype=np.float32)
    imp = (impv * (1.0 - eye)).astype(np.float32)
    coeff = (0.1 / (1.0 + imp)) * (1.0 - eye)

    # B_b = Wt @ (G_b @ W_out);  (G_b@W_out)[g*HD+e,:] = sum_i c[b,i,g] Wo4[i,e,:]
    Wo4 = W_out.reshape(H, HD, D)
    Bmats = []
    for b in range(B):
        Mb = np.einsum('ig,ied->ged', coeff[b], Wo4).reshape(D, D)
        Bmats.append(np.ascontiguousarray(Wt @ Mb))

    # ---- launch 2: main pass ----
    cs_all = (np.arange(S, dtype=np.float32) + 1.0) / S
    gamma_bc = np.ascontiguousarray(np.broadcast_to(gamma, (128, D)))
    beta_bc = np.ascontiguousarray(np.broadcast_to(beta, (128, D)))
    bout_bc = np.ascontiguousarray(np.broadcast_to(b_out, (128, D)))
    in_maps = []
    for c in range(NCORES):
        b, half = divmod(c, 2)
        cs_c = cs_all[half * TOK:(half + 1) * TOK]
        cs_t = np.ascontiguousarray(cs_c.reshape(NT, 128).T)
        in_maps.append({
            "xT": shards[c], "A": A, "Bb": Bmats[b], "cs": cs_t,
            "gamma_bc": gamma_bc, "beta_bc": beta_bc, "bout_bc": bout_bc,
        })
    triv = (not b_out.any(), bool((gamma == 1.0).all()), not beta.any())
    main_nc = _get_nc("main", triv=triv)
    # LN-output invariant for a corruption check: sampled rows must have
    # mean ~ mean(beta) and second moment ~ mean(gamma^2 + beta^2), loosely.
    m_lo = 0.2 * float((gamma ** 2 + beta ** 2).mean()) + 1e-6
    m_hi = 5.0 * float((gamma ** 2 + beta ** 2).mean()) + 1e-6
    rows = np.linspace(0, TOK - 1, 16).astype(int)
    last_exc = None
    for attempt in range(3):
        try:
            res2 = run_bass_kernel_spmd(main_nc, in_maps, CORE_IDS)
        except Exception as e:
            last_exc = e
            time.sleep(2.0)
            continue
        ok = True
        for c in range(NCORES):
            yr = res2.results[c]["y"][rows]
            if not np.isfinite(yr).all():
                ok = False
                break
            m2 = float((yr.astype(np.float64) ** 2).mean())
            if not (m_lo <= m2 <= m_hi):
                ok = False
                break
        if ok:
            break
    else:
        raise RuntimeError(
            f"main launch failed integrity check 3x (last exc: {last_exc})")

    y = np.empty((B, S, D), dtype=np.float32)
    for c in range(NCORES):
        b, half = divmod(c, 2)
        y[b, half * TOK:(half + 1) * TOK, :] = res2.results[c]["y"]
    return y, imp


# revision 16
# speedup vs baseline: 1.1488x; 1.1090x over previous
"""Trainium2 Bass kernel for nn_DongTaiBaGuaZhen.

Reference math (B=4, S=8192, D=1024, HD=128, H=8 heads, P=32):
    heads[h]  = (x @ W_proj[h]) * cos(pi*freqs[h])
    hm        = heads.mean(S)                  -> tiny "impedance" net -> imp, coeff (B,8,8)
    mix[i]    = sum_g coeff[b,i,g] * heads[g]
    out_heads = heads + ((s+1)/S) * mix
    y         = LayerNorm(concat(out_heads) @ W_out + b_out + x) * gamma + beta

Key algebra: the S-mean commutes with the projection, so hm needs only
xbar = mean_s(x).  The whole main pass collapses to

    y_pre[b,s] = x[b,s] @ (Wt @ W_out + I)  +  cs(s) * (x[b,s] @ Wt @ G_b @ W_out) + b_out
    y = LN(y_pre) * gamma + beta

with Wt = concat_h(W_proj[h]*cos(pi*freqs[h])) (D,D) and G_b = coeff[b].T (x) I_HD.

Plan (8 NeuronCores, tokens (B*S = 32768) sharded contiguously, 4096/core,
so each core holds half of one batch's sequence):
  Launch 1 (device): per-core column sums of x  -> host: xbar_b, tiny net (exact
            fp32 numpy), imp output, A' = Wt@W_out + I, B_b = Wt@(G_b@W_out).
  Launch 2 (device): per 128-token group: u = x@A' and v = x@B_b via float32r
            matmuls (PSUM f32 accumulation), epilogue
            ypre = u + cs*v + b_out; LayerNorm via bn_stats; *gamma + beta; DMA out.
"""

import math
import time

import numpy as np

import concourse.bacc as bacc
import concourse.mybir as mybir
import concourse.tile as tile
from concourse.bass_utils import run_bass_kernel_spmd

B, S, D, HD, P, H = 4, 8192, 1024, 128, 32, 8
EPS_LN = 1e-5
NCORES = 8
TOK = (B * S) // NCORES      # 4096 tokens per core
NT = TOK // 128              # 32 token groups of 128
CH = 512                     # free-dim width of x^T chunk tiles
NCH = TOK // CH              # 8 chunks
NDB = D // 128               # 8 din blocks
F32 = mybir.dt.float32
F32R = mybir.dt.float32r
CORE_IDS = list(range(NCORES))

_BUILT = {}
LAST_EXEC_NS = {}


def _build_stats_nc(tok=TOK, reps=1):
    """Launch 1: colsum[p, dblk] = sum_t xT[dblk*128+p, t] per core."""
    nc = bacc.Bacc("TRN2", target_bir_lowering=False, debug=False,
                   num_devices=NCORES)
    xT = nc.dram_tensor("xT", [D, tok], F32, kind="ExternalInput").ap()
    colsum = nc.dram_tensor("colsum", [128, NDB], F32,
                            kind="ExternalOutput").ap()
    with tile.TileContext(nc) as tc:
        with (
            tc.tile_pool(name="sb", bufs=1) as pool,
            tc.tile_pool(name="acc", bufs=1) as accp,
        ):
            half = tok // 2
            acc = accp.tile([128, NDB, 2], F32, tag="acc")
            fin = accp.tile([128, NDB], F32, tag="fin")
            for _rep in range(reps):
                for dblk in range(NDB):
                    for hh in range(2):
                        t = pool.tile([128, half], F32, tag=f"x{dblk}_{hh}")
                        nc.sync.dma_start(
                            t[:], xT[dblk * 128:(dblk + 1) * 128,
                                     hh * half:(hh + 1) * half])
                        nc.vector.tensor_reduce(
                            acc[:, dblk, hh:hh + 1], t[:],
                            axis=mybir.AxisListType.X, op=mybir.AluOpType.add)
            nc.vector.tensor_reduce(
                fin[:], acc[:], axis=mybir.AxisListType.X,
                op=mybir.AluOpType.add)
            nc.sync.dma_start(colsum[:], fin[:])
    nc.compile()
    return nc


def _build_main_nc(tok=TOK, reps=1, triv=(False, False, False),
                   order="interleave"):
    """Launch 2: y = LN(x@A' + cs*(x@B) + b_out) * gamma + beta.

    triv = (bout_is_zero, gamma_is_one, beta_is_zero): skip the matching
    epilogue ops (DVE is nearly co-critical with PE otherwise).  The host
    checks the actual input values and picks the variant; the general path
    still handles arbitrary b_out/gamma/beta.
    """
    nc = bacc.Bacc("TRN2", target_bir_lowering=False, debug=False,
                   num_devices=NCORES)
    nt = tok // 128
    xT = nc.dram_tensor("xT", [D, tok], F32R, kind="ExternalInput").ap()
    Ad = nc.dram_tensor("A", [D, D], F32R, kind="ExternalInput").ap()
    Bd = nc.dram_tensor("Bb", [D, D], F32R, kind="ExternalInput").ap()
    csd = nc.dram_tensor("cs", [128, nt], F32, kind="ExternalInput").ap()
    gbc = nc.dram_tensor("gamma_bc", [128, D], F32, kind="ExternalInput").ap()
    bbc = nc.dram_tensor("beta_bc", [128, D], F32, kind="ExternalInput").ap()
    obc = nc.dram_tensor("bout_bc", [128, D], F32, kind="ExternalInput").ap()
    yd = nc.dram_tensor("y", [tok, D], F32, kind="ExternalOutput").ap()

    with tile.TileContext(nc) as tc:
        with (
            tc.tile_pool(name="wp", bufs=1) as wp,
            tc.tile_pool(name="cp", bufs=1) as cp,
            tc.tile_pool(name="xp", bufs=2) as xp,
            tc.tile_pool(name="pp", bufs=2, space="PSUM") as pp,
            tc.tile_pool(name="op", bufs=3) as op,
            tc.tile_pool(name="sp", bufs=4) as sp,
        ):
            skip_bout, skip_gamma, skip_beta = triv
            At, Bt = [], []
            first_x = []
            for dblk in range(NDB):
                a = wp.tile([128, D], F32R, tag=f"A{dblk}")
                nc.sync.dma_start(a[:], Ad[dblk * 128:(dblk + 1) * 128, :])
                At.append(a)
                xt = xp.tile([128, CH], F32R, tag=f"x{dblk}")
                nc.sync.dma_start(xt[:], xT[dblk * 128:(dblk + 1) * 128,
                                            0:CH])
                first_x.append(xt)
            cst = cp.tile([128, nt], F32, tag="cs")
            nc.sync.dma_start(cst[:], csd[:])
            gt = bt = ot = None
            if not skip_gamma:
                gt = cp.tile([128, D], F32, tag="g")
                nc.sync.dma_start(gt[:], gbc[:])
            if not skip_beta:
                bt = cp.tile([128, D], F32, tag="b")
                nc.sync.dma_start(bt[:], bbc[:])
            if not skip_bout:
                ot = cp.tile([128, D], F32, tag="o")
                nc.sync.dma_start(ot[:], obc[:])
            eps = cp.tile([128, 1], F32, tag="eps")
            nc.vector.memset(eps[:], float(EPS_LN))

            xtiles = None
            for _rep in range(reps):
             for g in range(nt):
                c, sub = divmod(g, CH // 128)
                if sub == 0:
                    if c == 0 and first_x is not None:
                        xtiles = first_x
                        first_x = None
                    else:
                        xtiles = []
                        for dblk in range(NDB):
                            xt = xp.tile([128, CH], F32R, tag=f"x{dblk}")
                            nc.sync.dma_start(
                                xt[:], xT[dblk * 128:(dblk + 1) * 128,
                                          c * CH:(c + 1) * CH])
                            xtiles.append(xt)
                if not Bt:
                    # B loads after chunk-0 x so u-matmuls start sooner
                    for dblk in range(NDB):
                        b_ = wp.tile([128, D], F32R, tag=f"B{dblk}")
                        nc.sync.dma_start(
                            b_[:], Bd[dblk * 128:(dblk + 1) * 128, :])
                        Bt.append(b_)
                u = pp.tile([128, D], F32, tag="u")
                v = pp.tile([128, D], F32, tag="v")
                if order == "interleave":
                    for dblk in range(NDB):
                        lhs = xtiles[dblk][:, sub * 128:(sub + 1) * 128]
                        st = dblk == 0
                        sp_ = dblk == NDB - 1
                        nc.tensor.matmul(u[:, 0:512], lhs,
                                         At[dblk][:, 0:512],
                                         start=st, stop=sp_)
                        nc.tensor.matmul(u[:, 512:1024], lhs,
                                         At[dblk][:, 512:1024],
                                         start=st, stop=sp_)
                        nc.tensor.matmul(v[:, 0:512], lhs,
                                         Bt[dblk][:, 0:512],
                                         start=st, stop=sp_)
                        nc.tensor.matmul(v[:, 512:1024], lhs,
                                         Bt[dblk][:, 512:1024],
                                         start=st, stop=sp_)
                else:
                    for wt, ps_ in ((At, u), (Bt, v)):
                        for dblk in range(NDB):
                            lhs = xtiles[dblk][:, sub * 128:(sub + 1) * 128]
                            st = dblk == 0
                            sp_ = dblk == NDB - 1
                            nc.tensor.matmul(ps_[:, 0:512], lhs,
                                             wt[dblk][:, 0:512],
                                             start=st, stop=sp_)
                            nc.tensor.matmul(ps_[:, 512:1024], lhs,
                                             wt[dblk][:, 512:1024],
                                             start=st, stop=sp_)

                # epilogue: ypre = u + cs*v + b_out ; LN ; *gamma + beta
                tmp = op.tile([128, D], F32, tag="tmp")
                nc.scalar.activation(tmp[:], v[:],
                                     mybir.ActivationFunctionType.Copy,
                                     scale=cst[:, g:g + 1])
                ypre = op.tile([128, D], F32, tag="ypre")
                nc.vector.tensor_tensor(ypre[:], tmp[:], u[:],
                                        op=mybir.AluOpType.add)
                if not skip_bout:
                    nc.vector.tensor_tensor(ypre[:], ypre[:], ot[:],
                                            op=mybir.AluOpType.add)
                st6 = sp.tile([128, 2, 6], F32, tag="st6")
                nc.vector.bn_stats(st6[:, 0, :], ypre[:, 0:512])
                nc.vector.bn_stats(st6[:, 1, :], ypre[:, 512:1024])
                mv = sp.tile([128, 2], F32, tag="mv")
                nc.vector.bn_aggr(mv[:], st6[:])
                mean_ap = mv[:, 0:1]
                std = sp.tile([128, 1], F32, tag="std")
                nc.scalar.activation(std[:], mv[:, 1:2],
                                     mybir.ActivationFunctionType.Sqrt,
                                     bias=eps[:])
                rstd = sp.tile([128, 1], F32, tag="rstd")
                nc.vector.reciprocal(rstd[:], std[:])
                yn = op.tile([128, D], F32, tag="yn")
                nc.vector.tensor_scalar(yn[:], ypre[:], mean_ap, rstd[:],
                                        op0=mybir.AluOpType.subtract,
                                        op1=mybir.AluOpType.mult)
                yo = yn
                if not skip_gamma:
                    yg = op.tile([128, D], F32, tag="yg")
                    nc.vector.tensor_tensor(yg[:], yo[:], gt[:],
                                            op=mybir.AluOpType.mult)
                    yo = yg
                if not skip_beta:
                    yb = op.tile([128, D], F32, tag="yb")
                    nc.vector.tensor_tensor(yb[:], yo[:], bt[:],
                                            op=mybir.AluOpType.add)
                    yo = yb
                nc.sync.dma_start(yd[g * 128:(g + 1) * 128, :], yo[:])
    nc.compile()
    return nc


def _get_nc(name, tok=TOK, triv=(False, False, False)):
    key = (name, tok, triv)
    if key not in _BUILT:
        if name == "stats":
            _BUILT[key] = _build_stats_nc(tok)
        else:
            _BUILT[key] = _build_main_nc(tok, triv=triv)
    return _BUILT[key]


def _erf(x):
    return np.vectorize(math.erf)(x).astype(np.float32)


def kernel(**inputs):
    x = np.asarray(inputs["x"], dtype=np.float32)
    W_proj = np.asarray(inputs["W_proj"], dtype=np.float32)
    freqs = np.asarray(inputs["freqs"], dtype=np.float32)
    W_pol = np.asarray(inputs["W_pol"], dtype=np.float32)
    b_pol = np.asarray(inputs["b_pol"], dtype=np.float32)
    W_imp1 = np.asarray(inputs["W_imp1"], dtype=np.float32)
    b_imp1 = np.asarray(inputs["b_imp1"], dtype=np.float32)
    W_imp2 = np.asarray(inputs["W_imp2"], dtype=np.float32)
    b_imp2 = np.asarray(inputs["b_imp2"], dtype=np.float32)
    W_out = np.asarray(inputs["W_out"], dtype=np.float32)
    b_out = np.asarray(inputs["b_out"], dtype=np.float32)
    gamma = np.asarray(inputs["gamma"], dtype=np.float32)
    beta = np.asarray(inputs["beta"], dtype=np.float32)

    # Wt[din, h*HD+e] = W_proj[h, din, e] * cos(pi*freqs[h, e])
    Wt = (W_proj * np.cos(np.pi * freqs)[:, None, :]).transpose(1, 0, 2)
    Wt = np.ascontiguousarray(Wt.reshape(D, D))
    A = Wt @ W_out + np.eye(D, dtype=np.float32)

    # per-core token shards: core c -> batch c//2, half c%2 (contiguous 4096)
    shards = []
    for c in range(NCORES):
        b, half = divmod(c, 2)
        shards.append(np.ascontiguousarray(
            x[b, half * TOK:(half + 1) * TOK, :].T))

    # ---- launch 1: column sums -> xbar ----
    stats_nc = _get_nc("stats")
    # exact host sums of 4 sampled dims as a corruption check
    chk_dims = [0, 257, 514, 1023]
    chk = {c: shards[c][chk_dims, :].sum(axis=1) for c in range(NCORES)}
    last_exc = None
    for attempt in range(3):
        try:
            res1 = run_bass_kernel_spmd(
                stats_nc, [{"xT": s} for s in shards], CORE_IDS)
            colsums = [res1.results[c]["colsum"].T.reshape(D)
                       for c in range(NCORES)]
        except Exception as e:
            last_exc = e
            time.sleep(2.0)
            continue
        ok = all(
            np.allclose(colsums[c][chk_dims], chk[c], atol=2e-2)
            for c in range(NCORES))
        if ok:
            break
    else:
        raise RuntimeError(
            f"stats launch failed integrity check 3x (last exc: {last_exc})")
    xbar = np.stack([(colsums[2 * b] + colsums[2 * b + 1]) / S
                     for b in range(B)])  # (B, D)

    # ---- host: tiny impedance net (exact fp32) ----
    hmc = xbar @ Wt                       # (B, D): hm[b, h*HD+e]
    hm = hmc.reshape(B, H, HD)
    pv = np.tanh(np.einsum('bhd,hdp->bhp', hm, W_pol) + b_pol[None])
    nrm = np.maximum(np.linalg.norm(pv, axis=-1, keepdims=True), 1e-12)
    pm = pv / nrm
    dp = np.einsum('bhp,bgp->bhg', pm, pm)
    zz = dp[..., None] * W_imp1[0] + b_imp1
    z = (0.5 * zz * (1.0 + _erf(zz / np.sqrt(2.0)))).astype(np.float32)
    pre = np.einsum('bhgk,ko->bhgo', z, W_imp2)[..., 0] + b_imp2[0]
    impv = (np.log1p(np.exp(-np.abs(pre))) + np.maximum(pre, 0.0))
    eye = np.eye(H, dtype=np.float32)
    imp = (impv * (1.0 - eye)).astype(np.float32)
    coeff = (0.1 / (1.0 + imp)) * (1.0 - eye)

    # B_b = Wt @ (G_b @ W_out);  (G_b@W_out)[g*HD+e,:] = sum_i c[b,i,g] Wo4[i,e,:]
    Wo4 = W_out.reshape(H, HD, D)
    Bmats = []
    for b in range(B):
        Mb = np.einsum('ig,ied->ged', coeff[b], Wo4).reshape(D, D)
        Bmats.append(np.ascontiguousarray(Wt @ Mb))

    # ---- launch 2: main pass ----
    cs_all = (np.arange(S, dtype=np.float32) + 1.0) / S
    gamma_bc = np.ascontiguousarray(np.broadcast_to(gamma, (128, D)))
    beta_bc = np.ascontiguousarray(np.broadcast_to(beta, (128, D)))
    bout_bc = np.ascontiguousarray(np.broadcast_to(b_out, (128, D)))
    in_maps = []
    for c in range(NCORES):
        b, half = divmod(c, 2)
        cs_c = cs_all[half * TOK:(half + 1) * TOK]
        cs_t = np.ascontiguousarray(cs_c.reshape(NT, 128).T)
        in_maps.append({
            "xT": shards[c], "A": A, "Bb": Bmats[b], "cs": cs_t,
            "gamma_bc": gamma_bc, "beta_bc": beta_bc, "bout_bc": bout_bc,
        })
    triv = (not b_out.any(), bool((gamma == 1.0).all()), not beta.any())
    main_nc = _get_nc("main", triv=triv)
    # LN-output invariant for a corruption check: sampled rows must have
    # mean ~ mean(beta) and second moment ~ mean(gamma^2 + beta^2), loosely.
    m_lo = 0.2 * float((gamma ** 2 + beta ** 2).mean()) + 1e-6
    m_hi = 5.0 * float((gamma ** 2 + beta ** 2).mean()) + 1e-6
    rows = np.linspace(0, TOK - 1, 16).astype(int)
    last_exc = None
    for attempt in range(3):
        try:
            res2 = run_bass_kernel_spmd(main_nc, in_maps, CORE_IDS)
        except Exception as e:
            last_exc = e
            time.sleep(2.0)
            continue
        ok = True
        for c in range(NCORES):
            yr = res2.results[c]["y"][rows]
            if not np.isfinite(yr).all():
                ok = False
                break
            m2 = float((yr.astype(np.float64) ** 2).mean())
            if not (m_lo <= m2 <= m_hi):
                ok = False
                break
        if ok:
            break
    else:
        raise RuntimeError(
            f"main launch failed integrity check 3x (last exc: {last_exc})")

    y = np.empty((B, S, D), dtype=np.float32)
    for c in range(NCORES):
        b, half = divmod(c, 2)
        y[b, half * TOK:(half + 1) * TOK, :] = res2.results[c]["y"]
    return y, imp


# revision 17
# speedup vs baseline: 1.1774x; 1.0249x over previous
"""Trainium2 Bass kernel for nn_DongTaiBaGuaZhen.

Reference math (B=4, S=8192, D=1024, HD=128, H=8 heads, P=32):
    heads[h]  = (x @ W_proj[h]) * cos(pi*freqs[h])
    hm        = heads.mean(S)                  -> tiny "impedance" net -> imp, coeff (B,8,8)
    mix[i]    = sum_g coeff[b,i,g] * heads[g]
    out_heads = heads + ((s+1)/S) * mix
    y         = LayerNorm(concat(out_heads) @ W_out + b_out + x) * gamma + beta

Key algebra: the S-mean commutes with the projection, so hm needs only
xbar = mean_s(x).  The whole main pass collapses to

    y_pre[b,s] = x[b,s] @ (Wt @ W_out + I)  +  cs(s) * (x[b,s] @ Wt @ G_b @ W_out) + b_out
    y = LN(y_pre) * gamma + beta

with Wt = concat_h(W_proj[h]*cos(pi*freqs[h])) (D,D) and G_b = coeff[b].T (x) I_HD.

Plan (8 NeuronCores, tokens (B*S = 32768) sharded contiguously, 4096/core,
so each core holds half of one batch's sequence):
  Launch 1 (device): per-core column sums of x  -> host: xbar_b, tiny net (exact
            fp32 numpy), imp output, A' = Wt@W_out + I, B_b = Wt@(G_b@W_out).
  Launch 2 (device): per 128-token group: u = x@A' and v = x@B_b via float32r
            matmuls (PSUM f32 accumulation), epilogue
            ypre = u + cs*v + b_out; LayerNorm via bn_stats; *gamma + beta; DMA out.
"""

import math
import time

import numpy as np

import concourse.bacc as bacc
import concourse.mybir as mybir
import concourse.tile as tile
from concourse.bass_utils import run_bass_kernel_spmd

B, S, D, HD, P, H = 4, 8192, 1024, 128, 32, 8
EPS_LN = 1e-5
NCORES = 8
TOK = (B * S) // NCORES      # 4096 tokens per core
NT = TOK // 128              # 32 token groups of 128
CH = 512                     # free-dim width of x^T chunk tiles
NCH = TOK // CH              # 8 chunks
NDB = D // 128               # 8 din blocks
F32 = mybir.dt.float32
F32R = mybir.dt.float32r
CORE_IDS = list(range(NCORES))

_BUILT = {}
LAST_EXEC_NS = {}


def _build_stats_nc(tok=TOK, reps=1):
    """Launch 1: colsum[p, dblk] = sum_t xT[dblk*128+p, t] per core."""
    nc = bacc.Bacc("TRN2", target_bir_lowering=False, debug=False,
                   num_devices=NCORES)
    xT = nc.dram_tensor("xT", [D, tok], F32, kind="ExternalInput").ap()
    colsum = nc.dram_tensor("colsum", [128, NDB], F32,
                            kind="ExternalOutput").ap()
    with tile.TileContext(nc) as tc:
        with (
            tc.tile_pool(name="sb", bufs=1) as pool,
            tc.tile_pool(name="acc", bufs=1) as accp,
        ):
            half = tok // 2
            acc = accp.tile([128, NDB, 2], F32, tag="acc")
            fin = accp.tile([128, NDB], F32, tag="fin")
            for _rep in range(reps):
                for dblk in range(NDB):
                    for hh in range(2):
                        t = pool.tile([128, half], F32, tag=f"x{dblk}_{hh}")
                        nc.sync.dma_start(
                            t[:], xT[dblk * 128:(dblk + 1) * 128,
                                     hh * half:(hh + 1) * half])
                        nc.vector.tensor_reduce(
                            acc[:, dblk, hh:hh + 1], t[:],
                            axis=mybir.AxisListType.X, op=mybir.AluOpType.add)
            nc.vector.tensor_reduce(
                fin[:], acc[:], axis=mybir.AxisListType.X,
                op=mybir.AluOpType.add)
            nc.sync.dma_start(colsum[:], fin[:])
    nc.compile()
    return nc


def _build_main_nc(tok=TOK, reps=1, triv=(False, False, False),
                   order="interleave", ch=CH, xbufs=2):
    """Launch 2: y = LN(x@A' + cs*(x@B) + b_out) * gamma + beta.

    triv = (bout_is_zero, gamma_is_one, beta_is_zero): skip the matching
    epilogue ops (DVE is nearly co-critical with PE otherwise).  The host
    checks the actual input values and picks the variant; the general path
    still handles arbitrary b_out/gamma/beta.
    """
    nc = bacc.Bacc("TRN2", target_bir_lowering=False, debug=False,
                   num_devices=NCORES)
    nt = tok // 128
    xT = nc.dram_tensor("xT", [D, tok], F32R, kind="ExternalInput").ap()
    Ad = nc.dram_tensor("A", [D, D], F32R, kind="ExternalInput").ap()
    Bd = nc.dram_tensor("Bb", [D, D], F32R, kind="ExternalInput").ap()
    csd = nc.dram_tensor("cs", [128, nt], F32, kind="ExternalInput").ap()
    gbc = nc.dram_tensor("gamma_bc", [128, D], F32, kind="ExternalInput").ap()
    bbc = nc.dram_tensor("beta_bc", [128, D], F32, kind="ExternalInput").ap()
    obc = nc.dram_tensor("bout_bc", [128, D], F32, kind="ExternalInput").ap()
    yd = nc.dram_tensor("y", [tok, D], F32, kind="ExternalOutput").ap()

    with tile.TileContext(nc) as tc:
        with (
            tc.tile_pool(name="wp", bufs=1) as wp,
            tc.tile_pool(name="cp", bufs=1) as cp,
            tc.tile_pool(name="xp", bufs=xbufs) as xp,
            tc.tile_pool(name="pp", bufs=2, space="PSUM") as pp,
            tc.tile_pool(name="op", bufs=3) as op,
            tc.tile_pool(name="sp", bufs=4) as sp,
        ):
            skip_bout, skip_gamma, skip_beta = triv
            At, Bt = [], []
            first_x = []
            for dblk in range(NDB):
                a = wp.tile([128, D], F32R, tag=f"A{dblk}")
                nc.sync.dma_start(a[:], Ad[dblk * 128:(dblk + 1) * 128, :])
                At.append(a)
                xt = xp.tile([128, ch], F32R, tag=f"x{dblk}")
                nc.sync.dma_start(xt[:], xT[dblk * 128:(dblk + 1) * 128,
                                            0:ch])
                first_x.append(xt)
            cst = cp.tile([128, nt], F32, tag="cs")
            nc.sync.dma_start(cst[:], csd[:])
            gt = bt = ot = None
            if not skip_gamma:
                gt = cp.tile([128, D], F32, tag="g")
                nc.sync.dma_start(gt[:], gbc[:])
            if not skip_beta:
                bt = cp.tile([128, D], F32, tag="b")
                nc.sync.dma_start(bt[:], bbc[:])
            if not skip_bout:
                ot = cp.tile([128, D], F32, tag="o")
                nc.sync.dma_start(ot[:], obc[:])
            eps = cp.tile([128, 1], F32, tag="eps")
            nc.vector.memset(eps[:], float(EPS_LN))

            xtiles = None
            for _rep in range(reps):
             for g in range(nt):
                c, sub = divmod(g, ch // 128)
                if sub == 0:
                    if c == 0 and first_x is not None:
                        xtiles = first_x
                        first_x = None
                    else:
                        xtiles = []
                        for dblk in range(NDB):
                            xt = xp.tile([128, ch], F32R, tag=f"x{dblk}")
                            nc.sync.dma_start(
                                xt[:], xT[dblk * 128:(dblk + 1) * 128,
                                          c * ch:(c + 1) * ch])
                            xtiles.append(xt)
                if not Bt:
                    # B loads after chunk-0 x so u-matmuls start sooner
                    for dblk in range(NDB):
                        b_ = wp.tile([128, D], F32R, tag=f"B{dblk}")
                        nc.sync.dma_start(
                            b_[:], Bd[dblk * 128:(dblk + 1) * 128, :])
                        Bt.append(b_)
                u = pp.tile([128, D], F32, tag="u")
                v = pp.tile([128, D], F32, tag="v")
                if order == "interleave":
                    for dblk in range(NDB):
                        lhs = xtiles[dblk][:, sub * 128:(sub + 1) * 128]
                        st = dblk == 0
                        sp_ = dblk == NDB - 1
                        nc.tensor.matmul(u[:, 0:512], lhs,
                                         At[dblk][:, 0:512],
                                         start=st, stop=sp_)
                        nc.tensor.matmul(u[:, 512:1024], lhs,
                                         At[dblk][:, 512:1024],
                                         start=st, stop=sp_)
                        nc.tensor.matmul(v[:, 0:512], lhs,
                                         Bt[dblk][:, 0:512],
                                         start=st, stop=sp_)
                        nc.tensor.matmul(v[:, 512:1024], lhs,
                                         Bt[dblk][:, 512:1024],
                                         start=st, stop=sp_)
                else:
                    for wt, ps_ in ((At, u), (Bt, v)):
                        for dblk in range(NDB):
                            lhs = xtiles[dblk][:, sub * 128:(sub + 1) * 128]
                            st = dblk == 0
                            sp_ = dblk == NDB - 1
                            nc.tensor.matmul(ps_[:, 0:512], lhs,
                                             wt[dblk][:, 0:512],
                                             start=st, stop=sp_)
                            nc.tensor.matmul(ps_[:, 512:1024], lhs,
                                             wt[dblk][:, 512:1024],
                                             start=st, stop=sp_)

                # epilogue: ypre = u + cs*v + b_out ; LN ; *gamma + beta
                tmp = op.tile([128, D], F32, tag="tmp")
                nc.scalar.activation(tmp[:], v[:],
                                     mybir.ActivationFunctionType.Copy,
                                     scale=cst[:, g:g + 1])
                ypre = op.tile([128, D], F32, tag="ypre")
                nc.vector.tensor_tensor(ypre[:], tmp[:], u[:],
                                        op=mybir.AluOpType.add)
                if not skip_bout:
                    nc.vector.tensor_tensor(ypre[:], ypre[:], ot[:],
                                            op=mybir.AluOpType.add)
                st6 = sp.tile([128, 2, 6], F32, tag="st6")
                nc.vector.bn_stats(st6[:, 0, :], ypre[:, 0:512])
                nc.vector.bn_stats(st6[:, 1, :], ypre[:, 512:1024])
                mv = sp.tile([128, 2], F32, tag="mv")
                nc.vector.bn_aggr(mv[:], st6[:])
                mean_ap = mv[:, 0:1]
                std = sp.tile([128, 1], F32, tag="std")
                nc.scalar.activation(std[:], mv[:, 1:2],
                                     mybir.ActivationFunctionType.Sqrt,
                                     bias=eps[:])
                rstd = sp.tile([128, 1], F32, tag="rstd")
                nc.vector.reciprocal(rstd[:], std[:])
                yn = op.tile([128, D], F32, tag="yn")
                nc.vector.tensor_scalar(yn[:], ypre[:], mean_ap, rstd[:],
                                        op0=mybir.AluOpType.subtract,
                                        op1=mybir.AluOpType.mult)
                yo = yn
                if not skip_gamma:
                    yg = op.tile([128, D], F32, tag="yg")
                    nc.vector.tensor_tensor(yg[:], yo[:], gt[:],
                                            op=mybir.AluOpType.mult)
                    yo = yg
                if not skip_beta:
                    yb = op.tile([128, D], F32, tag="yb")
                    nc.vector.tensor_tensor(yb[:], yo[:], bt[:],
                                            op=mybir.AluOpType.add)
                    yo = yb
                nc.sync.dma_start(yd[g * 128:(g + 1) * 128, :], yo[:])
    nc.compile()
    return nc


def _get_nc(name, tok=TOK, triv=(False, False, False)):
    key = (name, tok, triv)
    if key not in _BUILT:
        if name == "stats":
            _BUILT[key] = _build_stats_nc(tok)
        else:
            _BUILT[key] = _build_main_nc(tok, triv=triv)
    return _BUILT[key]


def _erf(x):
    return np.vectorize(math.erf)(x).astype(np.float32)


def kernel(**inputs):
    x = np.asarray(inputs["x"], dtype=np.float32)
    W_proj = np.asarray(inputs["W_proj"], dtype=np.float32)
    freqs = np.asarray(inputs["freqs"], dtype=np.float32)
    W_pol = np.asarray(inputs["W_pol"], dtype=np.float32)
    b_pol = np.asarray(inputs["b_pol"], dtype=np.float32)
    W_imp1 = np.asarray(inputs["W_imp1"], dtype=np.float32)
    b_imp1 = np.asarray(inputs["b_imp1"], dtype=np.float32)
    W_imp2 = np.asarray(inputs["W_imp2"], dtype=np.float32)
    b_imp2 = np.asarray(inputs["b_imp2"], dtype=np.float32)
    W_out = np.asarray(inputs["W_out"], dtype=np.float32)
    b_out = np.asarray(inputs["b_out"], dtype=np.float32)
    gamma = np.asarray(inputs["gamma"], dtype=np.float32)
    beta = np.asarray(inputs["beta"], dtype=np.float32)

    # Wt[din, h*HD+e] = W_proj[h, din, e] * cos(pi*freqs[h, e])
    Wt = (W_proj * np.cos(np.pi * freqs)[:, None, :]).transpose(1, 0, 2)
    Wt = np.ascontiguousarray(Wt.reshape(D, D))
    A = Wt @ W_out + np.eye(D, dtype=np.float32)

    # per-core token shards: core c -> batch c//2, half c%2 (contiguous 4096)
    shards = []
    for c in range(NCORES):
        b, half = divmod(c, 2)
        shards.append(np.ascontiguousarray(
            x[b, half * TOK:(half + 1) * TOK, :].T))

    # ---- launch 1: column sums -> xbar ----
    stats_nc = _get_nc("stats")
    # exact host sums of 4 sampled dims as a corruption check
    chk_dims = [0, 257, 514, 1023]
    chk = {c: shards[c][chk_dims, :].sum(axis=1) for c in range(NCORES)}
    last_exc = None
    for attempt in range(3):
        try:
            res1 = run_bass_kernel_spmd(
                stats_nc, [{"xT": s} for s in shards], CORE_IDS)
            colsums = [res1.results[c]["colsum"].T.reshape(D)
                       for c in range(NCORES)]
        except Exception as e:
            last_exc = e
            time.sleep(2.0)
            continue
        ok = all(
            np.allclose(colsums[c][chk_dims], chk[c], atol=2e-2)
            for c in range(NCORES))
        if ok:
            break
    else:
        raise RuntimeError(
            f"stats launch failed integrity check 3x (last exc: {last_exc})")
    xbar = np.stack([(colsums[2 * b] + colsums[2 * b + 1]) / S
                     for b in range(B)])  # (B, D)

    # ---- host: tiny impedance net (exact fp32) ----
    hmc = xbar @ Wt                       # (B, D): hm[b, h*HD+e]
    hm = hmc.reshape(B, H, HD)
    pv = np.tanh(np.einsum('bhd,hdp->bhp', hm, W_pol) + b_pol[None])
    nrm = np.maximum(np.linalg.norm(pv, axis=-1, keepdims=True), 1e-12)
    pm = pv / nrm
    dp = np.einsum('bhp,bgp->bhg', pm, pm)
    zz = dp[..., None] * W_imp1[0] + b_imp1
    z = (0.5 * zz * (1.0 + _erf(zz / np.sqrt(2.0)))).astype(np.float32)
    pre = np.einsum('bhgk,ko->bhgo', z, W_imp2)[..., 0] + b_imp2[0]
    impv = (np.log1p(np.exp(-np.abs(pre))) + np.maximum(pre, 0.0))
    eye = np.eye(H, dtype=np.float32)
    imp = (impv * (1.0 - eye)).astype(np.float32)
    coeff = (0.1 / (1.0 + imp)) * (1.0 - eye)

    # B_b = Wt @ (G_b @ W_out);  (G_b@W_out)[g*HD+e,:] = sum_i c[b,i,g] Wo4[i,e,:]
    Wo4 = W_out.reshape(H, HD, D)
    Bmats = []
    for b in range(B):
        Mb = np.einsum('ig,ied->ged', coeff[b], Wo4).reshape(D, D)
        Bmats.append(np.ascontiguousarray(Wt @ Mb))

    # ---- launch 2: main pass ----
    cs_all = (np.arange(S, dtype=np.float32) + 1.0) / S
    gamma_bc = np.ascontiguousarray(np.broadcast_to(gamma, (128, D)))
    beta_bc = np.ascontiguousarray(np.broadcast_to(beta, (128, D)))
    bout_bc = np.ascontiguousarray(np.broadcast_to(b_out, (128, D)))
    in_maps = []
    for c in range(NCORES):
        b, half = divmod(c, 2)
        cs_c = cs_all[half * TOK:(half + 1) * TOK]
        cs_t = np.ascontiguousarray(cs_c.reshape(NT, 128).T)
        in_maps.append({
            "xT": shards[c], "A": A, "Bb": Bmats[b], "cs": cs_t,
            "gamma_bc": gamma_bc, "beta_bc": beta_bc, "bout_bc": bout_bc,
        })
    triv = (not b_out.any(), bool((gamma == 1.0).all()), not beta.any())
    main_nc = _get_nc("main", triv=triv)
    # LN-output invariant for a corruption check: sampled rows must have
    # mean ~ mean(beta) and second moment ~ mean(gamma^2 + beta^2), loosely.
    m_lo = 0.2 * float((gamma ** 2 + beta ** 2).mean()) + 1e-6
    m_hi = 5.0 * float((gamma ** 2 + beta ** 2).mean()) + 1e-6
    rows = np.linspace(0, TOK - 1, 16).astype(int)
    last_exc = None
    for attempt in range(3):
        try:
            res2 = run_bass_kernel_spmd(main_nc, in_maps, CORE_IDS)
        except Exception as e:
            last_exc = e
            time.sleep(2.0)
            continue
        ok = True
        for c in range(NCORES):
            yr = res2.results[c]["y"][rows]
            if not np.isfinite(yr).all():
                ok = False
                break
            m2 = float((yr.astype(np.float64) ** 2).mean())
            if not (m_lo <= m2 <= m_hi):
                ok = False
                break
        if ok:
            break
    else:
        raise RuntimeError(
            f"main launch failed integrity check 3x (last exc: {last_exc})")

    y = np.empty((B, S, D), dtype=np.float32)
    for c in range(NCORES):
        b, half = divmod(c, 2)
        y[b, half * TOK:(half + 1) * TOK, :] = res2.results[c]["y"]
    return y, imp
